# revision 3
# baseline (speedup 1.0000x reference)
"""CircularMaxPool2d (disk stencil, radius 5, reflect padding) on 8 TRN2 NeuronCores.

Input x: [8, 1, 2048, 2048] f32. Data-parallel: core c processes batch c.

Algorithm (exact fp32): decompose the disk mask by rows. For radius 5 the
disk rows are: dy=0 -> 11-wide, |dy| in {1,2,3} -> 9-wide, |dy|=4 -> 7-wide,
|dy|=5 -> 1-wide. So

  out[i,j] = max( h5[i,j], max_{|d|<=3} h4[i+d,j], h3[i-4,j], h3[i+4,j],
                  x[i-5,j], x[i+5,j] )

where hk = horizontal (2k+1)-wide running max of x. Horizontal maxes are
built with a shared doubling ladder (s1=2,s2=4,s3=6-wide), 6 DVE passes;
vertical combination takes 8 more passes, all pure free-dim ops.

Layout: partition p holds rows [16p, 16p+16). Vertical neighbor rows are
exchanged via partition-shifted SBUF->SBUF DMAs into halo slots of extended
tiles (DVE cannot read partition-shifted operands). W is reflect-padded on
the host; H reflection is folded into the halo fills of partitions 0/127.
"""

import sys

sys.path.insert(0, "/opt/trn_rl_repo")

import numpy as np

H = 2048
W = 2048
RAD = 5
WP = W + 2 * RAD  # 2058 host-padded width
NB = 8  # column bands
WB = W // NB  # 256 output cols per band
WH = WB + 2 * RAD  # 266 input cols per band
P = 128
G = 16  # rows per partition
N_CORES = 8

_CACHE = {}


def _build():
    import concourse.bacc as bacc
    import concourse.tile as tile
    import concourse.mybir as mybir

    f32 = mybir.dt.float32
    MAX = mybir.AluOpType.max

    nc = bacc.Bacc("TRN2", target_bir_lowering=False, debug=False, num_devices=N_CORES)
    xin = nc.dram_tensor("xin", [H, WP], f32, kind="ExternalInput").ap()
    yout = nc.dram_tensor("yout", [H, W], f32, kind="ExternalOutput").ap()
    xin_r = xin.rearrange("(p k) w -> p k w", k=G)  # [128, 16, 2058]
    yout_r = yout.rearrange("(p k) w -> p k w", k=G)  # [128, 16, 2048]

    with tile.TileContext(nc) as tc:
        with (
            tc.tile_pool(name="xx", bufs=2) as p_xx,
            tc.tile_pool(name="ladA", bufs=1) as p_a,
            tc.tile_pool(name="ladB", bufs=1) as p_b,
            tc.tile_pool(name="h4x", bufs=1) as p_h4,
            tc.tile_pool(name="h3x", bufs=1) as p_h3,
            tc.tile_pool(name="acc", bufs=2) as p_acc,
        ):
            for b in range(NB):
                jb = b * WB

                # ---- load x band, extended to 26 rows/partition (rows 16p-5..16p+20)
                xx = p_xx.tile([P, 26, WH], f32, tag="xx")
                nc.sync.dma_start(xx[:, 5:21, :], xin_r[:, :, jb : jb + WH])
                nc.sync.dma_start(xx[1:P, 0:5, :], xx[0 : P - 1, 16:21, :])
                nc.sync.dma_start(xx[0 : P - 1, 21:26, :], xx[1:P, 5:10, :])
                # H-reflection at image top/bottom: row -k -> k, row 2047+k -> 2047-k
                # (per-row DMAs: reversed-order block DMAs corrupt data on HW)
                for s in range(5):
                    nc.sync.dma_start(
                        xx[0:1, s : s + 1, :], xin_r[0:1, 5 - s : 6 - s, jb : jb + WH]
                    )
                    nc.sync.dma_start(
                        xx[P - 1 : P, 21 + s : 22 + s, :],
                        xin_r[P - 1 : P, 14 - s : 15 - s, jb : jb + WH],
                    )

                # ---- horizontal ladder on the 16 owned rows
                xc = xx[:, 5:21, :]
                s1 = p_a.tile([P, G, WH], f32, tag="A")
                nc.vector.tensor_tensor(
                    s1[:, :, 0:265], xc[:, :, 0:265], xc[:, :, 1:266], op=MAX
                )
                s2 = p_b.tile([P, G, WH], f32, tag="B")
                nc.vector.tensor_tensor(
                    s2[:, :, 0:263], s1[:, :, 0:263], s1[:, :, 2:265], op=MAX
                )
                s3 = p_a.tile([P, G, WH], f32, tag="A")
                nc.vector.tensor_tensor(
                    s3[:, :, 0:261], s2[:, :, 0:261], s2[:, :, 2:263], op=MAX
                )
                # h3 (7-wide, used at dy=+-4), into 24-slot extended tile (center 4..19)
                h3x = p_h3.tile([P, 24, WB], f32, tag="h3x")
                nc.vector.tensor_tensor(
                    h3x[:, 4:20, :], s2[:, :, 2 : 2 + WB], s2[:, :, 5 : 5 + WB], op=MAX
                )
                # h5 (11-wide, dy=0) straight into the accumulator
                acc = p_acc.tile([P, G, WB], f32, tag="acc")
                nc.vector.tensor_tensor(
                    acc[:, :, :], s3[:, :, 0:WB], s3[:, :, 5 : 5 + WB], op=MAX
                )
                # h4 (9-wide, |dy|<=3), into 22-slot extended tile (center 3..18)
                h4x = p_h4.tile([P, 22, WB], f32, tag="h4x")
                nc.vector.tensor_tensor(
                    h4x[:, 3:19, :], s3[:, :, 1 : 1 + WB], s3[:, :, 4 : 4 + WB], op=MAX
                )

                # ---- halo fills (vertical neighbor rows from adjacent partitions)
                nc.sync.dma_start(h4x[1:P, 0:3, :], h4x[0 : P - 1, 16:19, :])
                nc.sync.dma_start(h4x[0 : P - 1, 19:22, :], h4x[1:P, 3:6, :])
                for s in range(3):
                    nc.sync.dma_start(h4x[0:1, s : s + 1, :], h4x[0:1, 6 - s : 7 - s, :])
                    nc.sync.dma_start(
                        h4x[P - 1 : P, 19 + s : 20 + s, :],
                        h4x[P - 1 : P, 17 - s : 18 - s, :],
                    )
                nc.sync.dma_start(h3x[1:P, 0:4, :], h3x[0 : P - 1, 16:20, :])
                nc.sync.dma_start(h3x[0 : P - 1, 20:24, :], h3x[1:P, 4:8, :])
                for s in range(4):
                    nc.sync.dma_start(h3x[0:1, s : s + 1, :], h3x[0:1, 8 - s : 9 - s, :])
                    nc.sync.dma_start(
                        h3x[P - 1 : P, 20 + s : 21 + s, :],
                        h3x[P - 1 : P, 18 - s : 19 - s, :],
                    )

                # ---- vertical combine
                # t1[r] = max(h4[r], h4[r+1]) for r in -3..17  (slot = r+3)
                t1 = p_b.tile([P, 21, WB], f32, tag="B")
                nc.vector.tensor_tensor(t1[:, :, :], h4x[:, 0:21, :], h4x[:, 1:22, :], op=MAX)
                # t2[r] = max(t1[r], t1[r+2]) = max h4[r..r+3], r in -3..15 (slot = r+3)
                t2 = p_a.tile([P, 19, WB], f32, tag="A")
                nc.vector.tensor_tensor(t2[:, :, :], t1[:, 0:19, :], t1[:, 2:21, :], op=MAX)
                # acc = max(acc, t2[r-3], t2[r])  -> max over h4[r-3..r+3]
                nc.vector.tensor_tensor(acc[:], acc[:], t2[:, 0:16, :], op=MAX)
                nc.vector.tensor_tensor(acc[:], acc[:], t2[:, 3:19, :], op=MAX)
                # h3 taps at dy = -4, +4 (slot = r+-4 + 4)
                nc.vector.tensor_tensor(acc[:], acc[:], h3x[:, 0:16, :], op=MAX)
                nc.vector.tensor_tensor(acc[:], acc[:], h3x[:, 8:24, :], op=MAX)
                # x taps at dy = -5, +5 (xx slot = r+-5 + 5, col offset +5)
                nc.vector.tensor_tensor(
                    acc[:], acc[:], xx[:, 0:16, 5 : 5 + WB], op=MAX
                )
                nc.vector.tensor_tensor(
                    acc[:], acc[:], xx[:, 10:26, 5 : 5 + WB], op=MAX
                )

                nc.sync.dma_start(yout_r[:, :, jb : jb + WB], acc[:, :, :])

    nc.compile()
    return nc


def _get_nc():
    if "nc" not in _CACHE:
        _CACHE["nc"] = _build()
    return _CACHE["nc"]


def kernel(x, radius):
    from concourse.bass_utils import run_bass_kernel_spmd

    assert int(radius) == RAD
    x = np.asarray(x, dtype=np.float32)
    B, C = x.shape[0], x.shape[1]
    imgs = x.reshape(B * C, H, W)
    assert imgs.shape[0] == N_CORES

    # NaN -> sentinel (matches reference), reflect-pad W on host
    imgs = np.where(np.isnan(imgs), np.float32(-99.0), imgs)
    xp = np.pad(imgs, ((0, 0), (0, 0), (RAD, RAD)), mode="reflect")
    xp = np.ascontiguousarray(xp, dtype=np.float32)

    nc = _get_nc()
    in_maps = [{"xin": xp[c]} for c in range(N_CORES)]
    res = run_bass_kernel_spmd(nc, in_maps, core_ids=list(range(N_CORES)), trace=False)
    out = np.stack([res.results[c]["yout"] for c in range(N_CORES)])
    out = out.reshape(B, C, H, W)
    # sentinel positions back to NaN (matches reference)
    out = np.where(out == np.float32(-99.0), np.float32(np.nan), out)
    return out.astype(np.float32)


# revision 4
# speedup vs baseline: 1.0826x; 1.0826x over previous
"""CircularMaxPool2d (disk stencil, radius 5, reflect padding) on 8 TRN2 NeuronCores.

Input x: [8, 1, 2048, 2048] f32. Data-parallel: core c processes batch c.

Algorithm (exact fp32): decompose the disk mask by rows. For radius 5 the
disk rows are: dy=0 -> 11-wide, |dy| in {1,2,3} -> 9-wide, |dy|=4 -> 7-wide,
|dy|=5 -> 1-wide. So

  out[i,j] = max( h5[i,j], max_{|d|<=3} h4[i+d,j], h3[i-4,j], h3[i+4,j],
                  x[i-5,j], x[i+5,j] )

where hk = horizontal (2k+1)-wide running max of x. Horizontal maxes are
built with a shared doubling ladder (s1=2,s2=4,s3=6-wide), 6 DVE passes;
vertical combination takes 8 more passes, all pure free-dim ops.

Layout: partition p holds rows [16p, 16p+16). The input is packed on the
host into a blocked [band, 128, 26, 266] tensor with vertical halo rows and
reflect padding baked in, so every HBM load is fully contiguous. Vertical
neighbor rows of the on-chip h3/h4 tensors are exchanged via
partition-shifted SBUF->SBUF DMAs into halo slots of extended tiles (DVE
cannot read partition-shifted operands). Output is written blocked
[band, 128, 16, 256] and unscrambled on the host.
"""

import sys

sys.path.insert(0, "/opt/trn_rl_repo")

import numpy as np

H = 2048
W = 2048
RAD = 5
NB = 8  # column bands
WB = W // NB  # 256 output cols per band
WH = WB + 2 * RAD  # 266 input cols per band
P = 128
G = 16  # rows per partition
N_CORES = 8

_CACHE = {}


def _build():
    import concourse.bacc as bacc
    import concourse.tile as tile
    import concourse.mybir as mybir

    f32 = mybir.dt.float32
    MAX = mybir.AluOpType.max

    nc = bacc.Bacc("TRN2", target_bir_lowering=False, debug=False, num_devices=N_CORES)
    xin = nc.dram_tensor("xin", [NB, P, 26, WH], f32, kind="ExternalInput").ap()
    yout = nc.dram_tensor("yout", [NB, P, G, WB], f32, kind="ExternalOutput").ap()

    with tile.TileContext(nc) as tc:
        with (
            tc.tile_pool(name="xx", bufs=2) as p_xx,
            tc.tile_pool(name="ladA", bufs=1) as p_a,
            tc.tile_pool(name="ladB", bufs=1) as p_b,
            tc.tile_pool(name="h4x", bufs=1) as p_h4,
            tc.tile_pool(name="h3x", bufs=1) as p_h3,
            tc.tile_pool(name="acc", bufs=2) as p_acc,
        ):
            for b in range(NB):
                # ---- load packed x band (rows 16p-5..16p+20, halos pre-baked)
                xx = p_xx.tile([P, 26, WH], f32, tag="xx")
                nc.sync.dma_start(xx[:, :, :], xin[b])

                # ---- horizontal ladder on the 16 owned rows
                xc = xx[:, 5:21, :]
                s1 = p_a.tile([P, G, WH], f32, tag="A")
                nc.vector.tensor_tensor(
                    s1[:, :, 0:265], xc[:, :, 0:265], xc[:, :, 1:266], op=MAX
                )
                s2 = p_b.tile([P, G, WH], f32, tag="B")
                nc.vector.tensor_tensor(
                    s2[:, :, 0:263], s1[:, :, 0:263], s1[:, :, 2:265], op=MAX
                )
                s3 = p_a.tile([P, G, WH], f32, tag="A")
                nc.vector.tensor_tensor(
                    s3[:, :, 0:261], s2[:, :, 0:261], s2[:, :, 2:263], op=MAX
                )
                # h3 (7-wide, used at dy=+-4), into 24-slot extended tile (center 4..19)
                h3x = p_h3.tile([P, 24, WB], f32, tag="h3x")
                nc.vector.tensor_tensor(
                    h3x[:, 4:20, :], s2[:, :, 2 : 2 + WB], s2[:, :, 5 : 5 + WB], op=MAX
                )
                # h5 (11-wide, dy=0) straight into the accumulator
                acc = p_acc.tile([P, G, WB], f32, tag="acc")
                nc.vector.tensor_tensor(
                    acc[:, :, :], s3[:, :, 0:WB], s3[:, :, 5 : 5 + WB], op=MAX
                )
                # h4 (9-wide, |dy|<=3), into 22-slot extended tile (center 3..18)
                h4x = p_h4.tile([P, 22, WB], f32, tag="h4x")
                nc.vector.tensor_tensor(
                    h4x[:, 3:19, :], s3[:, :, 1 : 1 + WB], s3[:, :, 4 : 4 + WB], op=MAX
                )

                # ---- halo fills (vertical neighbor rows from adjacent partitions)
                # on the scalar (Activation) HWDGE queue so they overlap the
                # sync-queue band loads
                nc.scalar.dma_start(h4x[1:P, 0:3, :], h4x[0 : P - 1, 16:19, :])
                nc.scalar.dma_start(h4x[0 : P - 1, 19:22, :], h4x[1:P, 3:6, :])
                for s in range(3):
                    nc.scalar.dma_start(
                        h4x[0:1, s : s + 1, :], h4x[0:1, 6 - s : 7 - s, :]
                    )
                    nc.scalar.dma_start(
                        h4x[P - 1 : P, 19 + s : 20 + s, :],
                        h4x[P - 1 : P, 17 - s : 18 - s, :],
                    )
                nc.scalar.dma_start(h3x[1:P, 0:4, :], h3x[0 : P - 1, 16:20, :])
                nc.scalar.dma_start(h3x[0 : P - 1, 20:24, :], h3x[1:P, 4:8, :])
                for s in range(4):
                    nc.scalar.dma_start(
                        h3x[0:1, s : s + 1, :], h3x[0:1, 8 - s : 9 - s, :]
                    )
                    nc.scalar.dma_start(
                        h3x[P - 1 : P, 20 + s : 21 + s, :],
                        h3x[P - 1 : P, 18 - s : 19 - s, :],
                    )

                # ---- vertical combine
                # t1[r] = max(h4[r], h4[r+1]) for r in -3..17  (slot = r+3)
                t1 = p_b.tile([P, 21, WB], f32, tag="B")
                nc.vector.tensor_tensor(
                    t1[:, :, :], h4x[:, 0:21, :], h4x[:, 1:22, :], op=MAX
                )
                # t2[r] = max(t1[r], t1[r+2]) = max h4[r..r+3], r in -3..15 (slot = r+3)
                t2 = p_a.tile([P, 19, WB], f32, tag="A")
                nc.vector.tensor_tensor(
                    t2[:, :, :], t1[:, 0:19, :], t1[:, 2:21, :], op=MAX
                )
                # acc = max(acc, t2[r-3], t2[r])  -> max over h4[r-3..r+3]
                nc.vector.tensor_tensor(acc[:], acc[:], t2[:, 0:16, :], op=MAX)
                nc.vector.tensor_tensor(acc[:], acc[:], t2[:, 3:19, :], op=MAX)
                # h3 taps at dy = -4, +4 (slot = r-+4 + 4)
                nc.vector.tensor_tensor(acc[:], acc[:], h3x[:, 0:16, :], op=MAX)
                nc.vector.tensor_tensor(acc[:], acc[:], h3x[:, 8:24, :], op=MAX)
                # x taps at dy = -5, +5 (xx slot = r-+5 + 5, col offset +5)
                nc.vector.tensor_tensor(acc[:], acc[:], xx[:, 0:16, 5 : 5 + WB], op=MAX)
                nc.vector.tensor_tensor(
                    acc[:], acc[:], xx[:, 10:26, 5 : 5 + WB], op=MAX
                )

                nc.scalar.dma_start(yout[b], acc[:, :, :])

    nc.compile()
    return nc


def _get_nc():
    if "nc" not in _CACHE:
        _CACHE["nc"] = _build()
    return _CACHE["nc"]


def _pack_input(img):
    """[2048, 2048] -> [NB, 128, 26, 266] with reflect pad + vertical halos."""
    xpad = np.pad(img, ((RAD, RAD), (RAD, RAD)), mode="reflect")  # [2058, 2058]
    # windows over rows: wv[p, s, w] = xpad[16p + s, w], s in 0..25
    wv = np.lib.stride_tricks.sliding_window_view(xpad, 26, axis=0)  # [2033, 2058, 26]
    wv = wv[:: G].transpose(0, 2, 1)  # [128, 26, 2058]
    out = np.empty((NB, P, 26, WH), dtype=np.float32)
    for b in range(NB):
        out[b] = wv[:, :, b * WB : b * WB + WH]
    return out


def kernel(x, radius):
    from concourse.bass_utils import run_bass_kernel_spmd

    assert int(radius) == RAD
    x = np.asarray(x, dtype=np.float32)
    B, C = x.shape[0], x.shape[1]
    imgs = x.reshape(B * C, H, W)
    assert imgs.shape[0] == N_CORES

    imgs = np.where(np.isnan(imgs), np.float32(-99.0), imgs)

    nc = _get_nc()
    in_maps = [{"xin": _pack_input(imgs[c])} for c in range(N_CORES)]
    res = run_bass_kernel_spmd(nc, in_maps, core_ids=list(range(N_CORES)), trace=False)
    # unscramble blocked output: y[16p+k, 256b+j] = yblk[b, p, k, j]
    out = np.empty((N_CORES, H, W), dtype=np.float32)
    for c in range(N_CORES):
        yblk = res.results[c]["yout"]  # [NB, P, G, WB]
        out[c] = yblk.transpose(1, 2, 0, 3).reshape(H, W)
    out = out.reshape(B, C, H, W)
    out = np.where(out == np.float32(-99.0), np.float32(np.nan), out)
    return out.astype(np.float32)


# revision 5
# speedup vs baseline: 2.0368x; 1.8813x over previous
"""CircularMaxPool2d (disk stencil, radius 5, reflect padding) on 8 TRN2 NeuronCores.

Input x: [8, 1, 2048, 2048] f32. Data-parallel: core c processes batch c.

Algorithm (exact fp32): decompose the disk mask by rows. For radius 5 the
disk rows are: dy=0 -> 11-wide, |dy| in {1,2,3} -> 9-wide, |dy|=4 -> 7-wide,
|dy|=5 -> 1-wide. So

  out[i,j] = max( h5[i,j], max_{|d|<=3} h4[i+d,j], h3[i-4,j], h3[i+4,j],
                  x[i-5,j], x[i+5,j] )

where hk = horizontal (2k+1)-wide running max of x. Horizontal maxes are
built with a shared doubling ladder (s1=2,s2=4,s3=6-wide); vertical
combination uses a 2-level ladder for the h4 band plus direct taps. All ops
are free-dim DVE tensor_tensor maxes.

Layout: partition p holds rows [16p, 16p+16). The input is packed on the
host into a blocked [band, 128, 26, 266] tensor with 5 vertical halo rows
on each side and reflect padding baked in, so every HBM load is fully
contiguous and every vertical shift is a free-dim offset. The horizontal
ladder is computed on the halo rows too (DVE cannot read partition-shifted
operands, and partition-shifted SBUF->SBUF DMA is slow), so the kernel has
no on-device halo exchange at all. Output is written blocked
[band, 128, 16, 256] and unscrambled on the host.
"""

import sys

sys.path.insert(0, "/opt/trn_rl_repo")

import numpy as np

H = 2048
W = 2048
RAD = 5
NB = 8  # column bands
WB = W // NB  # 256 output cols per band
WH = WB + 2 * RAD  # 266 input cols per band
P = 128
G = 16  # rows per partition
N_CORES = 8

_CACHE = {}


def _build():
    import concourse.bacc as bacc
    import concourse.tile as tile
    import concourse.mybir as mybir

    f32 = mybir.dt.float32
    MAX = mybir.AluOpType.max

    nc = bacc.Bacc("TRN2", target_bir_lowering=False, debug=False, num_devices=N_CORES)
    xin = nc.dram_tensor("xin", [NB, P, 26, WH], f32, kind="ExternalInput").ap()
    yout = nc.dram_tensor("yout", [NB, P, G, WB], f32, kind="ExternalOutput").ap()

    with tile.TileContext(nc) as tc:
        with (
            tc.tile_pool(name="xx", bufs=2) as p_xx,
            tc.tile_pool(name="ladA", bufs=1) as p_a,
            tc.tile_pool(name="ladB", bufs=1) as p_b,
            tc.tile_pool(name="h4x", bufs=1) as p_h4,
            tc.tile_pool(name="h3x", bufs=1) as p_h3,
            tc.tile_pool(name="acc", bufs=2) as p_acc,
        ):
            for b in range(NB):
                # ---- load packed x band (rows 16p-5..16p+20, halos pre-baked)
                xx = p_xx.tile([P, 26, WH], f32, tag="xx")
                nc.sync.dma_start(xx[:, :, :], xin[b])

                # ---- horizontal ladder, on rows -4..19 (xx slots 1..24)
                # s1/s2 rows -4..19 (24 rows, slot = r+4); s3 rows -3..18 (22, slot = r+3)
                s1 = p_a.tile([P, 24, WH], f32, tag="A")
                nc.vector.tensor_tensor(
                    s1[:, :, 0:265], xx[:, 1:25, 0:265], xx[:, 1:25, 1:266], op=MAX
                )
                s2 = p_b.tile([P, 24, WH], f32, tag="B")
                nc.vector.tensor_tensor(
                    s2[:, :, 0:263], s1[:, :, 0:263], s1[:, :, 2:265], op=MAX
                )
                s3 = p_a.tile([P, 22, WH], f32, tag="A")
                nc.vector.tensor_tensor(
                    s3[:, :, 0:261], s2[:, 1:23, 0:261], s2[:, 1:23, 2:263], op=MAX
                )
                # h3 (7-wide, used at dy=+-4), rows -4..19 (slot = r+4)
                h3x = p_h3.tile([P, 24, WB], f32, tag="h3x")
                nc.vector.tensor_tensor(
                    h3x[:, :, :], s2[:, :, 2 : 2 + WB], s2[:, :, 5 : 5 + WB], op=MAX
                )
                # h5 (11-wide, dy=0) straight into the accumulator (rows 0..15)
                acc = p_acc.tile([P, G, WB], f32, tag="acc")
                nc.vector.tensor_tensor(
                    acc[:, :, :], s3[:, 3:19, 0:WB], s3[:, 3:19, 5 : 5 + WB], op=MAX
                )
                # h4 (9-wide, |dy|<=3), rows -3..18 (slot = r+3)
                h4x = p_h4.tile([P, 22, WB], f32, tag="h4x")
                nc.vector.tensor_tensor(
                    h4x[:, :, :], s3[:, :, 1 : 1 + WB], s3[:, :, 4 : 4 + WB], op=MAX
                )

                # ---- vertical combine
                # t1[r] = max(h4[r], h4[r+1]) for r in -3..17  (slot = r+3)
                t1 = p_b.tile([P, 21, WB], f32, tag="B")
                nc.vector.tensor_tensor(
                    t1[:, :, :], h4x[:, 0:21, :], h4x[:, 1:22, :], op=MAX
                )
                # t2[r] = max(t1[r], t1[r+2]) = max h4[r..r+3], r in -3..15 (slot = r+3)
                t2 = p_a.tile([P, 19, WB], f32, tag="A")
                nc.vector.tensor_tensor(
                    t2[:, :, :], t1[:, 0:19, :], t1[:, 2:21, :], op=MAX
                )
                # acc = max(acc, t2[r-3], t2[r])  -> max over h4[r-3..r+3]
                nc.vector.tensor_tensor(acc[:], acc[:], t2[:, 0:16, :], op=MAX)
                nc.vector.tensor_tensor(acc[:], acc[:], t2[:, 3:19, :], op=MAX)
                # h3 taps at dy = -4, +4 (slot = r-+4 + 4)
                nc.vector.tensor_tensor(acc[:], acc[:], h3x[:, 0:16, :], op=MAX)
                nc.vector.tensor_tensor(acc[:], acc[:], h3x[:, 8:24, :], op=MAX)
                # x taps at dy = -5, +5 (xx slot = r-+5 + 5, col offset +5)
                nc.vector.tensor_tensor(acc[:], acc[:], xx[:, 0:16, 5 : 5 + WB], op=MAX)
                nc.vector.tensor_tensor(
                    acc[:], acc[:], xx[:, 10:26, 5 : 5 + WB], op=MAX
                )

                nc.scalar.dma_start(yout[b], acc[:, :, :])

    nc.compile()
    return nc


def _get_nc():
    if "nc" not in _CACHE:
        _CACHE["nc"] = _build()
    return _CACHE["nc"]


def _pack_input(img):
    """[2048, 2048] -> [NB, 128, 26, 266] with reflect pad + vertical halos."""
    xpad = np.pad(img, ((RAD, RAD), (RAD, RAD)), mode="reflect")  # [2058, 2058]
    # windows over rows: wv[p, s, w] = xpad[16p + s, w], s in 0..25
    wv = np.lib.stride_tricks.sliding_window_view(xpad, 26, axis=0)  # [2033, 2058, 26]
    wv = wv[::G].transpose(0, 2, 1)  # [128, 26, 2058]
    out = np.empty((NB, P, 26, WH), dtype=np.float32)
    for b in range(NB):
        out[b] = wv[:, :, b * WB : b * WB + WH]
    return out


def kernel(x, radius):
    from concourse.bass_utils import run_bass_kernel_spmd

    assert int(radius) == RAD
    x = np.asarray(x, dtype=np.float32)
    B, C = x.shape[0], x.shape[1]
    imgs = x.reshape(B * C, H, W)
    assert imgs.shape[0] == N_CORES

    imgs = np.where(np.isnan(imgs), np.float32(-99.0), imgs)

    nc = _get_nc()
    in_maps = [{"xin": _pack_input(imgs[c])} for c in range(N_CORES)]
    res = run_bass_kernel_spmd(nc, in_maps, core_ids=list(range(N_CORES)), trace=False)
    # unscramble blocked output: y[16p+k, 256b+j] = yblk[b, p, k, j]
    out = np.empty((N_CORES, H, W), dtype=np.float32)
    for c in range(N_CORES):
        yblk = res.results[c]["yout"]  # [NB, P, G, WB]
        out[c] = yblk.transpose(1, 2, 0, 3).reshape(H, W)
    out = out.reshape(B, C, H, W)
    out = np.where(out == np.float32(-99.0), np.float32(np.nan), out)
    return out.astype(np.float32)


# revision 6
# speedup vs baseline: 2.1982x; 1.0793x over previous
"""CircularMaxPool2d (disk stencil, radius 5, reflect padding) on 8 TRN2 NeuronCores.

Input x: [8, 1, 2048, 2048] f32. Data-parallel: core c processes batch c.

Algorithm (exact fp32): decompose the disk mask by rows. For radius 5 the
disk rows are: dy=0 -> 11-wide, |dy| in {1,2,3} -> 9-wide, |dy|=4 -> 7-wide,
|dy|=5 -> 1-wide. So

  out[i,j] = max( h5[i,j], max_{|d|<=3} h4[i+d,j], h3[i-4,j], h3[i+4,j],
                  x[i-5,j], x[i+5,j] )

where hk = horizontal (2k+1)-wide running max of x. Horizontal maxes are
built with a shared doubling ladder (s1=2,s2=4,s3=6-wide); the vertical
combination uses a 2-level ladder for the h4 band plus direct taps. All ops
are free-dim DVE tensor_tensor maxes (fp32 tensor_tensor = 1 elem/cyc/lane;
this kernel is DVE-bound, DMA fully hidden).

Layout: each partition owns a (column-chunk, row-group) pair: G=32
consecutive rows x WB=128 columns. The input is packed on the host into a
blocked [superband, 128, G+10, WB+10] tensor with vertical halo rows and
reflect padding baked in, so every HBM load is fully contiguous and every
vertical shift is a free-dim offset. The horizontal ladder is computed on
the halo rows too (DVE cannot read partition-shifted operands, and
partition-shifted SBUF->SBUF DMA is slow ~22GB/s), so the kernel needs no
on-device halo exchange at all. Output is written blocked and unscrambled
on the host.
"""

import sys

sys.path.insert(0, "/opt/trn_rl_repo")

import numpy as np

H = 2048
W = 2048
RAD = 5
P = 128
G = 32  # rows per partition group
NG = H // G  # 64 row groups
NCHUNK = P // NG  # 2 column chunks per superband
WB = 128  # cols per chunk
WH = WB + 2 * RAD  # 138
NSB = W // (WB * NCHUNK)  # 8 superbands
XR = G + 2 * RAD  # 42 rows in x tile
N_CORES = 8

_CACHE = {}


def _build():
    import concourse.bacc as bacc
    import concourse.tile as tile
    import concourse.mybir as mybir

    f32 = mybir.dt.float32
    MAX = mybir.AluOpType.max

    nc = bacc.Bacc("TRN2", target_bir_lowering=False, debug=False, num_devices=N_CORES)
    xin = nc.dram_tensor("xin", [NSB, P, XR, WH], f32, kind="ExternalInput").ap()
    yout = nc.dram_tensor("yout", [NSB, P, G, WB], f32, kind="ExternalOutput").ap()

    with tile.TileContext(nc) as tc:
        with (
            tc.tile_pool(name="xx", bufs=2) as p_xx,
            tc.tile_pool(name="ladA", bufs=1) as p_a,
            tc.tile_pool(name="ladB", bufs=1) as p_b,
            tc.tile_pool(name="h4x", bufs=1) as p_h4,
            tc.tile_pool(name="h3x", bufs=1) as p_h3,
            tc.tile_pool(name="acc", bufs=2) as p_acc,
        ):
            for b in range(NSB):
                # ---- load packed x band (rows Gp-5..Gp+G+4, halos pre-baked)
                xx = p_xx.tile([P, XR, WH], f32, tag="xx")
                nc.sync.dma_start(xx[:, :, :], xin[b])

                # ---- horizontal ladder on rows -4..G+3 (xx slots 1..G+8)
                # s1/s2 rows -4..G+3 (G+8 rows, slot = r+4); s3 rows -3..G+2 (G+6, slot = r+3)
                s1 = p_a.tile([P, G + 8, WH], f32, tag="A")
                nc.vector.tensor_tensor(
                    s1[:, :, 0 : WH - 1],
                    xx[:, 1 : G + 9, 0 : WH - 1],
                    xx[:, 1 : G + 9, 1:WH],
                    op=MAX,
                )
                s2 = p_b.tile([P, G + 8, WH], f32, tag="B")
                nc.vector.tensor_tensor(
                    s2[:, :, 0 : WH - 3],
                    s1[:, :, 0 : WH - 3],
                    s1[:, :, 2 : WH - 1],
                    op=MAX,
                )
                s3 = p_a.tile([P, G + 6, WH], f32, tag="A")
                nc.vector.tensor_tensor(
                    s3[:, :, 0 : WH - 5],
                    s2[:, 1 : G + 7, 0 : WH - 5],
                    s2[:, 1 : G + 7, 2 : WH - 3],
                    op=MAX,
                )
                # h3 (7-wide, used at dy=+-4), rows -4..G+3 (slot = r+4)
                h3x = p_h3.tile([P, G + 8, WB], f32, tag="h3x")
                nc.vector.tensor_tensor(
                    h3x[:, :, :], s2[:, :, 2 : 2 + WB], s2[:, :, 5 : 5 + WB], op=MAX
                )
                # h5 (11-wide, dy=0) straight into the accumulator (rows 0..G-1)
                acc = p_acc.tile([P, G, WB], f32, tag="acc")
                nc.vector.tensor_tensor(
                    acc[:, :, :],
                    s3[:, 3 : G + 3, 0:WB],
                    s3[:, 3 : G + 3, 5 : 5 + WB],
                    op=MAX,
                )
                # h4 (9-wide, |dy|<=3), rows -3..G+2 (slot = r+3)
                h4x = p_h4.tile([P, G + 6, WB], f32, tag="h4x")
                nc.vector.tensor_tensor(
                    h4x[:, :, :], s3[:, :, 1 : 1 + WB], s3[:, :, 4 : 4 + WB], op=MAX
                )

                # ---- vertical combine
                # t1[r] = max(h4[r], h4[r+1]) for r in -3..G+1  (slot = r+3)
                t1 = p_b.tile([P, G + 5, WB], f32, tag="B")
                nc.vector.tensor_tensor(
                    t1[:, :, :], h4x[:, 0 : G + 5, :], h4x[:, 1 : G + 6, :], op=MAX
                )
                # t2[r] = max(t1[r], t1[r+2]) = max h4[r..r+3], r in -3..G-1 (slot = r+3)
                t2 = p_a.tile([P, G + 3, WB], f32, tag="A")
                nc.vector.tensor_tensor(
                    t2[:, :, :], t1[:, 0 : G + 3, :], t1[:, 2 : G + 5, :], op=MAX
                )
                # acc = max(acc, t2[r-3], t2[r])  -> max over h4[r-3..r+3]
                nc.vector.tensor_tensor(acc[:], acc[:], t2[:, 0:G, :], op=MAX)
                nc.vector.tensor_tensor(acc[:], acc[:], t2[:, 3 : G + 3, :], op=MAX)
                # h3 taps at dy = -4, +4 (slot = r-+4 + 4)
                nc.vector.tensor_tensor(acc[:], acc[:], h3x[:, 0:G, :], op=MAX)
                nc.vector.tensor_tensor(acc[:], acc[:], h3x[:, 8 : G + 8, :], op=MAX)
                # x taps at dy = -5, +5 (xx slot = r-+5 + 5, col offset +5)
                nc.vector.tensor_tensor(
                    acc[:], acc[:], xx[:, 0:G, 5 : 5 + WB], op=MAX
                )
                nc.vector.tensor_tensor(
                    acc[:], acc[:], xx[:, 10 : G + 10, 5 : 5 + WB], op=MAX
                )

                nc.scalar.dma_start(yout[b], acc[:, :, :])

    nc.compile()
    return nc


def _get_nc():
    if "nc" not in _CACHE:
        _CACHE["nc"] = _build()
    return _CACHE["nc"]


def _pack_input(img):
    """[2048, 2048] -> [NSB, 128, XR, WH] with reflect pad + vertical halos.

    Partition p of superband s holds rows [G*g-5, G*g+G+5) and cols
    [(NCHUNK*s + c)*WB - 5, ... + WB + 5) of the original image, where
    c = p // NG, g = p % NG (indices in reflect-padded coordinates).
    """
    xpad = np.pad(img, ((RAD, RAD), (RAD, RAD)), mode="reflect")  # [2058, 2058]
    wv = np.lib.stride_tricks.sliding_window_view(xpad, XR, axis=0)  # [H+11-XR, 2058, XR]
    wv = wv[::G].transpose(0, 2, 1)  # [NG, XR, 2058]
    out = np.empty((NSB, P, XR, WH), dtype=np.float32)
    for s in range(NSB):
        for c in range(NCHUNK):
            j0 = (NCHUNK * s + c) * WB
            out[s, c * NG : (c + 1) * NG] = wv[:, :, j0 : j0 + WH]
    return out


def _unpack_output(yblk):
    """[NSB, 128, G, WB] -> [2048, 2048]."""
    y = np.empty((H, W), dtype=np.float32)
    for s in range(NSB):
        for c in range(NCHUNK):
            j0 = (NCHUNK * s + c) * WB
            blk = yblk[s, c * NG : (c + 1) * NG]  # [NG, G, WB]
            y[:, j0 : j0 + WB] = blk.reshape(H, WB)
    return y


def kernel(x, radius):
    from concourse.bass_utils import run_bass_kernel_spmd

    assert int(radius) == RAD
    x = np.asarray(x, dtype=np.float32)
    B, C = x.shape[0], x.shape[1]
    imgs = x.reshape(B * C, H, W)
    assert imgs.shape[0] == N_CORES

    imgs = np.where(np.isnan(imgs), np.float32(-99.0), imgs)

    nc = _get_nc()
    in_maps = [{"xin": _pack_input(imgs[c])} for c in range(N_CORES)]
    res = run_bass_kernel_spmd(nc, in_maps, core_ids=list(range(N_CORES)), trace=False)
    out = np.empty((N_CORES, H, W), dtype=np.float32)
    for c in range(N_CORES):
        out[c] = _unpack_output(res.results[c]["yout"])
    out = out.reshape(B, C, H, W)
    out = np.where(out == np.float32(-99.0), np.float32(np.nan), out)
    return out.astype(np.float32)


# revision 7
# speedup vs baseline: 2.2693x; 1.0323x over previous
"""CircularMaxPool2d (disk stencil, radius 5, reflect padding) on 8 TRN2 NeuronCores.

Input x: [8, 1, 2048, 2048] f32. Data-parallel: core c processes batch c.

Algorithm (exact fp32): decompose the disk mask by rows. For radius 5 the
disk rows are: dy=0 -> 11-wide, |dy| in {1,2,3} -> 9-wide, |dy|=4 -> 7-wide,
|dy|=5 -> 1-wide. So

  out[i,j] = max( h5[i,j], max_{|d|<=3} h4[i+d,j], h3[i-4,j], h3[i+4,j],
                  x[i-5,j], x[i+5,j] )

where hk = horizontal (2k+1)-wide running max of x. Horizontal maxes are
built with a shared doubling ladder (s1=2,s2=4,s3=6-wide); the vertical
combination uses a 2-level ladder for the h4 band plus direct taps. All ops
are free-dim DVE tensor_tensor maxes (fp32 tensor_tensor = 1 elem/cyc/lane;
this kernel is DVE-bound, DMA fully hidden).

Layout: each partition owns a (column-chunk, row-group) pair: G=32
consecutive rows x WB=128 columns. The input is packed on the host into a
blocked [superband, 128, G+10, WB+10] tensor with vertical halo rows and
reflect padding baked in, so every HBM load is fully contiguous and every
vertical shift is a free-dim offset. The horizontal ladder is computed on
the halo rows too (DVE cannot read partition-shifted operands, and
partition-shifted SBUF->SBUF DMA is slow ~22GB/s), so the kernel needs no
on-device halo exchange at all. Output is written blocked and unscrambled
on the host.
"""

import sys

sys.path.insert(0, "/opt/trn_rl_repo")

import numpy as np

H = 2048
W = 2048
RAD = 5
P = 128
G = 64  # rows per partition group
NG = H // G  # row groups
NCHUNK = P // NG  # column chunks per superband
WB = 64  # cols per chunk
WH = WB + 2 * RAD  # 138
NSB = W // (WB * NCHUNK)  # 8 superbands
XR = G + 2 * RAD  # 42 rows in x tile
N_CORES = 8

_CACHE = {}


def _build():
    import concourse.bacc as bacc
    import concourse.tile as tile
    import concourse.mybir as mybir

    f32 = mybir.dt.float32
    MAX = mybir.AluOpType.max

    nc = bacc.Bacc("TRN2", target_bir_lowering=False, debug=False, num_devices=N_CORES)
    xin = nc.dram_tensor("xin", [NSB, P, XR, WH], f32, kind="ExternalInput").ap()
    yout = nc.dram_tensor("yout", [NSB, P, G, WB], f32, kind="ExternalOutput").ap()

    with tile.TileContext(nc) as tc:
        with (
            tc.tile_pool(name="xx", bufs=2) as p_xx,
            tc.tile_pool(name="ladA", bufs=1) as p_a,
            tc.tile_pool(name="ladB", bufs=1) as p_b,
            tc.tile_pool(name="h4x", bufs=1) as p_h4,
            tc.tile_pool(name="h3x", bufs=1) as p_h3,
            tc.tile_pool(name="acc", bufs=2) as p_acc,
        ):
            for b in range(NSB):
                # ---- load packed x band (rows Gp-5..Gp+G+4, halos pre-baked)
                xx = p_xx.tile([P, XR, WH], f32, tag="xx")
                nc.sync.dma_start(xx[:, :, :], xin[b])

                # ---- horizontal ladder on rows -4..G+3 (xx slots 1..G+8)
                # s1/s2 rows -4..G+3 (G+8 rows, slot = r+4); s3 rows -3..G+2 (G+6, slot = r+3)
                s1 = p_a.tile([P, G + 8, WH], f32, tag="A")
                nc.vector.tensor_tensor(
                    s1[:, :, 0 : WH - 1],
                    xx[:, 1 : G + 9, 0 : WH - 1],
                    xx[:, 1 : G + 9, 1:WH],
                    op=MAX,
                )
                s2 = p_b.tile([P, G + 8, WH], f32, tag="B")
                nc.vector.tensor_tensor(
                    s2[:, :, 0 : WH - 3],
                    s1[:, :, 0 : WH - 3],
                    s1[:, :, 2 : WH - 1],
                    op=MAX,
                )
                s3 = p_a.tile([P, G + 6, WH], f32, tag="A")
                nc.vector.tensor_tensor(
                    s3[:, :, 0 : WH - 5],
                    s2[:, 1 : G + 7, 0 : WH - 5],
                    s2[:, 1 : G + 7, 2 : WH - 3],
                    op=MAX,
                )
                # h3 (7-wide, used at dy=+-4), rows -4..G+3 (slot = r+4)
                h3x = p_h3.tile([P, G + 8, WB], f32, tag="h3x")
                nc.vector.tensor_tensor(
                    h3x[:, :, :], s2[:, :, 2 : 2 + WB], s2[:, :, 5 : 5 + WB], op=MAX
                )
                # h5 (11-wide, dy=0) straight into the accumulator (rows 0..G-1)
                acc = p_acc.tile([P, G, WB], f32, tag="acc")
                nc.vector.tensor_tensor(
                    acc[:, :, :],
                    s3[:, 3 : G + 3, 0:WB],
                    s3[:, 3 : G + 3, 5 : 5 + WB],
                    op=MAX,
                )
                # h4 (9-wide, |dy|<=3), rows -3..G+2 (slot = r+3)
                h4x = p_h4.tile([P, G + 6, WB], f32, tag="h4x")
                nc.vector.tensor_tensor(
                    h4x[:, :, :], s3[:, :, 1 : 1 + WB], s3[:, :, 4 : 4 + WB], op=MAX
                )

                # ---- vertical combine
                # t1[r] = max(h4[r], h4[r+1]) for r in -3..G+1  (slot = r+3)
                t1 = p_b.tile([P, G + 5, WB], f32, tag="B")
                nc.vector.tensor_tensor(
                    t1[:, :, :], h4x[:, 0 : G + 5, :], h4x[:, 1 : G + 6, :], op=MAX
                )
                # t2[r] = max(t1[r], t1[r+2]) = max h4[r..r+3], r in -3..G-1 (slot = r+3)
                t2 = p_a.tile([P, G + 3, WB], f32, tag="A")
                nc.vector.tensor_tensor(
                    t2[:, :, :], t1[:, 0 : G + 3, :], t1[:, 2 : G + 5, :], op=MAX
                )
                # acc = max(acc, t2[r-3], t2[r])  -> max over h4[r-3..r+3]
                nc.vector.tensor_tensor(acc[:], acc[:], t2[:, 0:G, :], op=MAX)
                nc.vector.tensor_tensor(acc[:], acc[:], t2[:, 3 : G + 3, :], op=MAX)
                # h3 taps at dy = -4, +4 (slot = r-+4 + 4)
                nc.vector.tensor_tensor(acc[:], acc[:], h3x[:, 0:G, :], op=MAX)
                nc.vector.tensor_tensor(acc[:], acc[:], h3x[:, 8 : G + 8, :], op=MAX)
                # x taps at dy = -5, +5 (xx slot = r-+5 + 5, col offset +5)
                nc.vector.tensor_tensor(
                    acc[:], acc[:], xx[:, 0:G, 5 : 5 + WB], op=MAX
                )
                nc.vector.tensor_tensor(
                    acc[:], acc[:], xx[:, 10 : G + 10, 5 : 5 + WB], op=MAX
                )

                nc.scalar.dma_start(yout[b], acc[:, :, :])

    nc.compile()
    return nc


def _get_nc():
    if "nc" not in _CACHE:
        _CACHE["nc"] = _build()
    return _CACHE["nc"]


def _pack_input(img):
    """[2048, 2048] -> [NSB, 128, XR, WH] with reflect pad + vertical halos.

    Partition p of superband s holds rows [G*g-5, G*g+G+5) and cols
    [(NCHUNK*s + c)*WB - 5, ... + WB + 5) of the original image, where
    c = p // NG, g = p % NG (indices in reflect-padded coordinates).
    """
    xpad = np.pad(img, ((RAD, RAD), (RAD, RAD)), mode="reflect")  # [2058, 2058]
    wv = np.lib.stride_tricks.sliding_window_view(xpad, XR, axis=0)  # [H+11-XR, 2058, XR]
    wv = wv[::G].transpose(0, 2, 1)  # [NG, XR, 2058]
    out = np.empty((NSB, P, XR, WH), dtype=np.float32)
    for s in range(NSB):
        for c in range(NCHUNK):
            j0 = (NCHUNK * s + c) * WB
            out[s, c * NG : (c + 1) * NG] = wv[:, :, j0 : j0 + WH]
    return out


def _unpack_output(yblk):
    """[NSB, 128, G, WB] -> [2048, 2048]."""
    y = np.empty((H, W), dtype=np.float32)
    for s in range(NSB):
        for c in range(NCHUNK):
            j0 = (NCHUNK * s + c) * WB
            blk = yblk[s, c * NG : (c + 1) * NG]  # [NG, G, WB]
            y[:, j0 : j0 + WB] = blk.reshape(H, WB)
    return y


def kernel(x, radius):
    from concourse.bass_utils import run_bass_kernel_spmd

    assert int(radius) == RAD
    x = np.asarray(x, dtype=np.float32)
    B, C = x.shape[0], x.shape[1]
    imgs = x.reshape(B * C, H, W)
    assert imgs.shape[0] == N_CORES

    imgs = np.where(np.isnan(imgs), np.float32(-99.0), imgs)

    nc = _get_nc()
    in_maps = [{"xin": _pack_input(imgs[c])} for c in range(N_CORES)]
    res = run_bass_kernel_spmd(nc, in_maps, core_ids=list(range(N_CORES)), trace=False)
    out = np.empty((N_CORES, H, W), dtype=np.float32)
    for c in range(N_CORES):
        out[c] = _unpack_output(res.results[c]["yout"])
    out = out.reshape(B, C, H, W)
    out = np.where(out == np.float32(-99.0), np.float32(np.nan), out)
    return out.astype(np.float32)


# revision 8
# speedup vs baseline: 2.2832x; 1.0062x over previous
"""CircularMaxPool2d (disk stencil, radius 5, reflect padding) on 8 TRN2 NeuronCores.

Input x: [8, 1, 2048, 2048] f32. Data-parallel: core c processes batch c.

Algorithm (exact fp32): decompose the disk mask by rows. For radius 5 the
disk rows are: dy=0 -> 11-wide, |dy| in {1,2,3} -> 9-wide, |dy|=4 -> 7-wide,
|dy|=5 -> 1-wide. So

  out[i,j] = max( h5[i,j], max_{|d|<=3} h4[i+d,j], h3[i-4,j], h3[i+4,j],
                  x[i-5,j], x[i+5,j] )

where hk = horizontal (2k+1)-wide running max of x. Horizontal maxes are
built with a shared doubling ladder (s1=2,s2=4,s3=6-wide); the vertical
combination uses a 2-level ladder for the h4 band plus direct taps. All ops
are free-dim DVE tensor_tensor maxes (fp32 tensor_tensor = 1 elem/cyc/lane;
this kernel is DVE-bound, DMA fully hidden).

Layout: each partition owns a (column-chunk, row-group) pair: G=32
consecutive rows x WB=128 columns. The input is packed on the host into a
blocked [superband, 128, G+10, WB+10] tensor with vertical halo rows and
reflect padding baked in, so every HBM load is fully contiguous and every
vertical shift is a free-dim offset. The horizontal ladder is computed on
the halo rows too (DVE cannot read partition-shifted operands, and
partition-shifted SBUF->SBUF DMA is slow ~22GB/s), so the kernel needs no
on-device halo exchange at all. Output is written blocked and unscrambled
on the host.
"""

import sys

sys.path.insert(0, "/opt/trn_rl_repo")

import numpy as np

H = 2048
W = 2048
RAD = 5
P = 128
G = 64  # rows per partition group
NG = H // G  # row groups
NCHUNK = P // NG  # column chunks per superband
WB = 64  # cols per chunk
WH = WB + 2 * RAD  # 138
NSB = W // (WB * NCHUNK)  # 8 superbands
XR = G + 2 * RAD  # 42 rows in x tile
N_CORES = 8

_CACHE = {}


def _build():
    import concourse.bacc as bacc
    import concourse.tile as tile
    import concourse.mybir as mybir

    f32 = mybir.dt.float32
    MAX = mybir.AluOpType.max

    nc = bacc.Bacc("TRN2", target_bir_lowering=False, debug=False, num_devices=N_CORES)
    xin = nc.dram_tensor("xin", [NSB, P, XR, WH], f32, kind="ExternalInput").ap()
    yout = nc.dram_tensor("yout", [NSB, P, G, WB], f32, kind="ExternalOutput").ap()

    with tile.TileContext(nc) as tc:
        with (
            tc.tile_pool(name="xx", bufs=2) as p_xx,
            tc.tile_pool(name="ladA", bufs=1) as p_a,
            tc.tile_pool(name="ladB", bufs=1) as p_b,
            tc.tile_pool(name="h4x", bufs=1) as p_h4,
            tc.tile_pool(name="h3x", bufs=1) as p_h3,
            tc.tile_pool(name="acc", bufs=2) as p_acc,
        ):
            for b in range(NSB):
                # ---- load packed x band (rows Gp-5..Gp+G+4, halos pre-baked)
                xx = p_xx.tile([P, XR, WH], f32, tag="xx")
                s1 = p_a.tile([P, G + 8, WH], f32, tag="A")
                if b == 0:
                    # split the cold-start load so the ladder starts after the
                    # first half lands (trims the pipeline ramp)
                    hs = XR // 2  # 37
                    nc.sync.dma_start(xx[:, 0:hs, :], xin[b][:, 0:hs, :])
                    nc.sync.dma_start(xx[:, hs:XR, :], xin[b][:, hs:XR, :])
                    nc.vector.tensor_tensor(
                        s1[:, 0 : hs - 1, 0 : WH - 1],
                        xx[:, 1:hs, 0 : WH - 1],
                        xx[:, 1:hs, 1:WH],
                        op=MAX,
                    )
                    nc.vector.tensor_tensor(
                        s1[:, hs - 1 : G + 8, 0 : WH - 1],
                        xx[:, hs : G + 9, 0 : WH - 1],
                        xx[:, hs : G + 9, 1:WH],
                        op=MAX,
                    )
                else:
                    nc.sync.dma_start(xx[:, :, :], xin[b])
                    # ---- horizontal ladder on rows -4..G+3 (xx slots 1..G+8)
                    # s1/s2 rows -4..G+3 (G+8, slot = r+4); s3 rows -3..G+2 (G+6, slot = r+3)
                    nc.vector.tensor_tensor(
                        s1[:, :, 0 : WH - 1],
                        xx[:, 1 : G + 9, 0 : WH - 1],
                        xx[:, 1 : G + 9, 1:WH],
                        op=MAX,
                    )
                s2 = p_b.tile([P, G + 8, WH], f32, tag="B")
                nc.vector.tensor_tensor(
                    s2[:, :, 0 : WH - 3],
                    s1[:, :, 0 : WH - 3],
                    s1[:, :, 2 : WH - 1],
                    op=MAX,
                )
                s3 = p_a.tile([P, G + 6, WH], f32, tag="A")
                nc.vector.tensor_tensor(
                    s3[:, :, 0 : WH - 5],
                    s2[:, 1 : G + 7, 0 : WH - 5],
                    s2[:, 1 : G + 7, 2 : WH - 3],
                    op=MAX,
                )
                # h3 (7-wide, used at dy=+-4), rows -4..G+3 (slot = r+4)
                h3x = p_h3.tile([P, G + 8, WB], f32, tag="h3x")
                nc.vector.tensor_tensor(
                    h3x[:, :, :], s2[:, :, 2 : 2 + WB], s2[:, :, 5 : 5 + WB], op=MAX
                )
                # h5 (11-wide, dy=0) straight into the accumulator (rows 0..G-1)
                acc = p_acc.tile([P, G, WB], f32, tag="acc")
                nc.vector.tensor_tensor(
                    acc[:, :, :],
                    s3[:, 3 : G + 3, 0:WB],
                    s3[:, 3 : G + 3, 5 : 5 + WB],
                    op=MAX,
                )
                # h4 (9-wide, |dy|<=3), rows -3..G+2 (slot = r+3)
                h4x = p_h4.tile([P, G + 6, WB], f32, tag="h4x")
                nc.vector.tensor_tensor(
                    h4x[:, :, :], s3[:, :, 1 : 1 + WB], s3[:, :, 4 : 4 + WB], op=MAX
                )

                # ---- vertical combine
                # t1[r] = max(h4[r], h4[r+1]) for r in -3..G+1  (slot = r+3)
                t1 = p_b.tile([P, G + 5, WB], f32, tag="B")
                nc.vector.tensor_tensor(
                    t1[:, :, :], h4x[:, 0 : G + 5, :], h4x[:, 1 : G + 6, :], op=MAX
                )
                # t2[r] = max(t1[r], t1[r+2]) = max h4[r..r+3], r in -3..G-1 (slot = r+3)
                t2 = p_a.tile([P, G + 3, WB], f32, tag="A")
                nc.vector.tensor_tensor(
                    t2[:, :, :], t1[:, 0 : G + 3, :], t1[:, 2 : G + 5, :], op=MAX
                )
                # acc = max(acc, t2[r-3], t2[r])  -> max over h4[r-3..r+3]
                nc.vector.tensor_tensor(acc[:], acc[:], t2[:, 0:G, :], op=MAX)
                nc.vector.tensor_tensor(acc[:], acc[:], t2[:, 3 : G + 3, :], op=MAX)
                # h3 taps at dy = -4, +4 (slot = r-+4 + 4)
                nc.vector.tensor_tensor(acc[:], acc[:], h3x[:, 0:G, :], op=MAX)
                nc.vector.tensor_tensor(acc[:], acc[:], h3x[:, 8 : G + 8, :], op=MAX)
                # x taps at dy = -5, +5 (xx slot = r-+5 + 5, col offset +5)
                nc.vector.tensor_tensor(
                    acc[:], acc[:], xx[:, 0:G, 5 : 5 + WB], op=MAX
                )
                nc.vector.tensor_tensor(
                    acc[:], acc[:], xx[:, 10 : G + 10, 5 : 5 + WB], op=MAX
                )

                nc.scalar.dma_start(yout[b], acc[:, :, :])

    nc.compile()
    return nc


def _get_nc():
    if "nc" not in _CACHE:
        _CACHE["nc"] = _build()
    return _CACHE["nc"]


def _pack_input(img):
    """[2048, 2048] -> [NSB, 128, XR, WH] with reflect pad + vertical halos.

    Partition p of superband s holds rows [G*g-5, G*g+G+5) and cols
    [(NCHUNK*s + c)*WB - 5, ... + WB + 5) of the original image, where
    c = p // NG, g = p % NG (indices in reflect-padded coordinates).
    """
    xpad = np.pad(img, ((RAD, RAD), (RAD, RAD)), mode="reflect")  # [2058, 2058]
    wv = np.lib.stride_tricks.sliding_window_view(xpad, XR, axis=0)  # [H+11-XR, 2058, XR]
    wv = wv[::G].transpose(0, 2, 1)  # [NG, XR, 2058]
    out = np.empty((NSB, P, XR, WH), dtype=np.float32)
    for s in range(NSB):
        for c in range(NCHUNK):
            j0 = (NCHUNK * s + c) * WB
            out[s, c * NG : (c + 1) * NG] = wv[:, :, j0 : j0 + WH]
    return out


def _unpack_output(yblk):
    """[NSB, 128, G, WB] -> [2048, 2048]."""
    y = np.empty((H, W), dtype=np.float32)
    for s in range(NSB):
        for c in range(NCHUNK):
            j0 = (NCHUNK * s + c) * WB
            blk = yblk[s, c * NG : (c + 1) * NG]  # [NG, G, WB]
            y[:, j0 : j0 + WB] = blk.reshape(H, WB)
    return y


def kernel(x, radius):
    from concourse.bass_utils import run_bass_kernel_spmd

    assert int(radius) == RAD
    x = np.asarray(x, dtype=np.float32)
    B, C = x.shape[0], x.shape[1]
    imgs = x.reshape(B * C, H, W)
    assert imgs.shape[0] == N_CORES

    imgs = np.where(np.isnan(imgs), np.float32(-99.0), imgs)

    nc = _get_nc()
    in_maps = [{"xin": _pack_input(imgs[c])} for c in range(N_CORES)]
    res = run_bass_kernel_spmd(nc, in_maps, core_ids=list(range(N_CORES)), trace=False)
    out = np.empty((N_CORES, H, W), dtype=np.float32)
    for c in range(N_CORES):
        out[c] = _unpack_output(res.results[c]["yout"])
    out = out.reshape(B, C, H, W)
    out = np.where(out == np.float32(-99.0), np.float32(np.nan), out)
    return out.astype(np.float32)
